# revision 28
# baseline (speedup 1.0000x reference)
"""Trainium2 Bass kernel for AnalyticalCatastropheDetector.

Strategy (8-core pure data parallel):
 - Host: transpose each batch shard to feature-major, stack two halves on
   the partition axis -> x2 [128, NCOLS].  All device DMAs are contiguous.
 - mm1: block-diag(W1,W1) stationary, x2 moving -> h1 stacked [128, N].
 - gelu on ACT (bias b1 fused), full 128 lanes.
 - mm2: g-tile stationary, block-diag Weff moving -> C row-major, where
   Weff = W2 @ Wh (folded, gelu-linear), columns permuted/duplicated into
   31 "blocks" ordered so head math uses contiguous mega-ops.
 - PSUM->SBUF copy deinterleaves C into block-major layout and adds the
   folded bias beta = b2 @ Wh + bh (one scalar_tensor_tensor per chunk).
 - Head math: DVE STT/TT chains + one mega Exp + 7 Tanh on ACT
   (sigmoid(v) = (1+tanh(v/2))/2, outputs scaled by 2, host divides).
   All ACT funcs in phase 2 live in the exp_and_others table set ->
   exactly one table switch after the gelu phase.
"""

import numpy as np
import os

MM_BF16 = os.environ.get("K_MM_BF16", "1") == "1"
HEAD_BF16 = os.environ.get("K_HEAD_BF16", "1") == "1"

B = 1_000_000
D = 64
NCORE = 8
BC = B // NCORE          # 125000 rows per core
HALF = BC // 2           # 62500
CHUNK = 1024             # x2 columns per chunk (= 2048 rows)
NCH = (HALF + CHUNK - 1) // CHUNK          # 62 chunks
NCOLS = NCH * CHUNK                        # 63488 (padded)
NB = 37                  # csb blocks: 30 copied + 7 computed Z
GPC = 2 * (CHUNK // 128)                   # 16 groups (of 128 rows) per chunk
NG = NCH * GPC                             # 992 groups per core
SUBS = [38, 24]                            # chunks per head sub-batch
assert sum(SUBS) == NCH

# Original coefficient indices (order in the reference's 29-col C):
# fold a=0 b=1 | cusp a=2 b=3 c=4 | swal a=5 b=6 c=7 d=8
# butt a=9 b=10 c=11 d=12 e=13 | hyp a=14 b=15 c=16 | ell a=19 b=20 c=21
# par a=24 b=25 c=26 d=27 e=28   (17,18,22,23 unused)
# Block table: (source_coeff, scale) per block position.
# pos 0-9:  plain-abs group; pos 10-16: square group; pos 17-30: raw group.
S3 = 3.0 ** (-1.0 / 3.0)
S10 = 10.0 ** (-1.0 / 3.0)
SB2 = 1.125 ** 0.5
# copied blocks 0-32 (ABS 0-8 | SQ 9-15 | RAW 16-29 | Zcopied 30-32);
# computed Z blocks 33-39 have no weff column.
BLOCKS = [
    (1, 1.0),            # 0  fold_b   (sig)
    (4, 1.0),            # 1  cusp_c   (sig)
    (8, 1.0),            # 2  swal_d   (sig)
    (13, 1.0),           # 3  butt_e   (sig)
    (14, 1.0),           # 4  hyp_a    (sig)
    (19, 1.0),           # 5  ell_a    (sig)
    (27, 1.0),           # 6  par_d    (sig)
    (28, 1.0),           # 7  par_e    (sig)
    (3, 3.0),            # 8  cusp_3b  -> |3b|
    (2, 1.0),            # 9  cusp_a   (sq)
    (5, 1.0),            # 10 swal_a   (sq)
    (9, S3),             # 11 butt_a3  (sq + raw cube factor)
    (10, SB2),           # 12 butt_bsq (sq)
    (14, S10),           # 13 hyp_a10  (sq + raw cube factor)
    (19, S10),           # 14 ell_a10  (sq + raw cube factor)
    (26, 0.5),           # 15 par_c2   (sq)
    (6, -4.0),           # 16 swal_bm4
    (9, 1.0),            # 17 butt_a
    (10, 1.0 / 6.0),     # 18 butt_b6
    (11, 1.0 / 3.0),     # 19 butt_c3
    (11, -1.5),          # 20 butt_c15
    (15, -2.7),          # 21 hyp_b27
    (16, 1.0),           # 22 hyp_c
    (20, 2.7),           # 23 ell_b27
    (21, 1.0),           # 24 ell_c
    (24, 1.0),           # 25 par_a
    (25, 1.0),           # 26 par_b
    (0, 1.0),            # 27 fold_a   (Z0)
    (7, 1.0),            # 28 swal_c   (Z1)
    (12, 1.0 / 3.0),     # 29 butt_d3  (Z2)
]
NCOPY = len(BLOCKS)  # 30
assert NCOPY == 30


def _build_bass(wsoft):
    import sys
    sys.path.insert(0, "/opt/trn_rl_repo")
    from concourse import bass, bacc, mybir
    from concourse.tile import TileContext

    F32 = mybir.dt.float32
    BF16 = mybir.dt.bfloat16
    MDT = BF16 if MM_BF16 else F32
    HDT = BF16 if HEAD_BF16 else F32
    AF = mybir.ActivationFunctionType
    OP = mybir.AluOpType

    nc = bacc.Bacc()
    x2 = nc.declare_dram_parameter("x2", [128, NCOLS], MDT, isOutput=False)
    w1bd = nc.declare_dram_parameter("w1bd", [128, 128], MDT, isOutput=False)
    b1bd = nc.declare_dram_parameter("b1bd", [128, 1], F32, isOutput=False)
    weffbd = nc.declare_dram_parameter("weffbd", [128, 2 * NCOPY], MDT, isOutput=False)
    betat = nc.declare_dram_parameter("betat", [128, GPC * NCOPY], F32, isOutput=False)
    cbias = nc.declare_dram_parameter("cbias", [128, 6], F32, isOutput=False)
    outr = nc.declare_dram_parameter("outr", [128, 7 * NG], HDT, isOutput=True)
    outt = nc.declare_dram_parameter("outt", [128, NG], HDT, isOutput=True)

    w = [float(v) for v in wsoft]

    with TileContext(nc) as tc:
        with (
            tc.tile_pool(name="const", bufs=1) as cpool,
            tc.tile_pool(name="xin", bufs=4) as xpool,
            tc.tile_pool(name="g", bufs=4) as gpool,
            tc.tile_pool(name="p1", bufs=2, space="PSUM") as p1pool,
            tc.tile_pool(name="pc", bufs=4, space="PSUM") as pcpool,
            tc.tile_pool(name="csb", bufs=2) as csbpool,
            tc.tile_pool(name="scr", bufs=1) as scr,
            tc.tile_pool(name="outp", bufs=2) as opool,
        ):
            w1t = cpool.tile([128, 128], MDT, tag="w1")
            nc.sync.dma_start(out=w1t[:], in_=w1bd[:])
            b1t = cpool.tile([128, 1], F32, tag="b1")
            nc.sync.dma_start(out=b1t[:], in_=b1bd[:])
            wet = cpool.tile([128, 2 * NCOPY], MDT, tag="weff")
            nc.sync.dma_start(out=wet[:], in_=weffbd[:])
            bet = cpool.tile([128, GPC * NCOPY], F32, tag="beta")
            nc.sync.dma_start(out=bet[:], in_=betat[:])
            cbt = cpool.tile([128, 6], F32, tag="cbias")
            nc.sync.dma_start(out=cbt[:], in_=cbias[:])
            CB = {0.5: cbt[:, 0:1], 0.25: cbt[:, 1:2], 0.15: cbt[:, 2:3],
                  0.1: cbt[:, 3:4], -0.25: cbt[:, 4:5], -0.5: cbt[:, 5:6]}

            roff = 0  # running col offset into outr
            toff = 0
            gch = 0   # global chunk index
            GPS = os.environ.get("K_GPS", "0") == "1"
            veng = nc.gpsimd if GPS else nc.vector
            for sb_chunks in SUBS:
                T = sb_chunks * GPC
                csb = csbpool.tile([128, NB * T], HDT, tag="csb")
                c3 = csb[:].rearrange("p (b t) -> p b t", b=NB)
                bsrc = bet[:].rearrange("p (b u) -> p b u", b=NCOPY)
                for ci in range(sb_chunks):
                    xoff = gch * CHUNK
                    xt = xpool.tile([128, CHUNK], MDT, tag="xt")
                    nc.sync.dma_start(out=xt[:], in_=x2[:, xoff:xoff + CHUNK])
                    p1 = p1pool.tile([128, CHUNK], F32, tag="p1")
                    for q5 in range(CHUNK // 512):
                        nc.tensor.matmul(p1[:, q5 * 512:(q5 + 1) * 512],
                                         w1t[:], xt[:, q5 * 512:(q5 + 1) * 512],
                                         start=True, stop=True)
                    gt = gpool.tile([128, CHUNK], MDT, tag="g")
                    nc.scalar.activation(gt[:], p1[:], AF.Gelu,
                                         bias=b1t[:, 0:1], scale=1.0)
                    pc = pcpool.tile([128, GPC * NCOPY], F32, tag="pc")
                    pv = pc[:].rearrange("p (b u) -> p b u", b=NCOPY)
                    for m in range(GPC // 2):
                        nc.tensor.matmul(
                            pv[:, :, m * 2:(m + 1) * 2].rearrange(
                                "p b s -> p s b"),
                            gt[:, m * 128:(m + 1) * 128],
                            wet[:], start=True, stop=True)
                    dst = c3[:, 0:NCOPY, ci * GPC:(ci + 1) * GPC]
                    nc.vector.scalar_tensor_tensor(
                        dst, pv, 1.0, bsrc, op0=OP.mult, op1=OP.add)
                    gch += 1

                # ---- head math over csb [128, NB*T] ----
                def blk(i):
                    return csb[:, i * T:(i + 1) * T]

                def nt(tagname, width=1):
                    return scr.tile([128, width * T], HDT, tag=tagname,
                                    name=tagname)

                stt = nc.vector.scalar_tensor_tensor
                tt = nc.vector.tensor_tensor
                ts = nc.vector.tensor_scalar

                X = [blk(i) for i in range(NB)]
                ABS = nt("ABS", 9)
                nc.scalar.activation(ABS[:], csb[:, 0:9 * T], AF.Abs)
                A = [ABS[:, i * T:(i + 1) * T] for i in range(9)]
                SQ = nt("SQ", 7)
                tt(SQ[:], csb[:, 9 * T:16 * T], csb[:, 9 * T:16 * T],
                   op=OP.mult)
                Q = [SQ[:, i * T:(i + 1) * T] for i in range(7)]

                # computed Z blocks (30-36) inside csb
                tt(X[30], Q[0], A[8], op=OP.subtract)       # cusp: a^2-3|b|
                tt(X[31], Q[1], X[16], op=OP.add)           # swal: a^2-4b
                t1 = nt("t1")
                tt(t1[:], X[17], X[18], op=OP.mult)         # ab/6
                tt(X[32], X[19], t1[:], op=OP.subtract)     # butt T2
                t2 = nt("t2")
                tt(t2[:], Q[2], X[11], op=OP.mult)          # a^3/3
                t3 = nt("t3")
                tt(t3[:], X[17], X[20], op=OP.mult)         # -1.5ac
                tt(t2[:], t2[:], t3[:], op=OP.add)
                tt(X[33], t2[:], Q[3], op=OP.add)           # butt T4
                t4 = nt("t4")
                tt(t4[:], Q[4], X[13], op=OP.mult)          # a^3/10
                t5 = nt("t5")
                tt(t5[:], X[21], X[22], op=OP.mult)         # -2.7bc
                tt(X[34], t4[:], t5[:], op=OP.add)          # hyp z
                tt(t4[:], Q[5], X[14], op=OP.mult)
                tt(t5[:], X[23], X[24], op=OP.mult)         # +2.7bc
                tt(X[35], t4[:], t5[:], op=OP.add)          # ell z
                tt(t1[:], X[25], X[26], op=OP.mult)         # ab
                tt(X[36], t1[:], Q[6], op=OP.subtract)      # par z

                Zv = csb[:, 27 * T:37 * T]
                ZA = nt("ZA", 10)
                stt(ZA[:], Zv, -1.0, Zv, op0=OP.mult, op1=OP.max)
                EX = nt("EX", 10)
                nc.scalar.activation(EX[:], ZA[:], AF.Exp, scale=-1.0)
                e = [EX[:, i * T:(i + 1) * T] for i in range(10)]
                # weighted E'_h = w_h * (product of exp factors)
                EW = nt("EW", 7)
                ew = [EW[:, i * T:(i + 1) * T] for i in range(7)]
                ts(ew[0], e[0], w[0], None, op0=OP.mult)
                ts(ew[1], e[3], w[1], None, op0=OP.mult)
                stt(ew[2], e[1], w[2], e[4], op0=OP.mult, op1=OP.mult)
                stt(ew[3], e[2], w[3], e[5], op0=OP.mult, op1=OP.mult)
                tt(ew[3], ew[3], e[6], op=OP.mult)
                ts(ew[4], e[7], w[4], None, op0=OP.mult)
                ts(ew[5], e[8], w[5], None, op0=OP.mult)
                ts(ew[6], e[9], w[6], None, op0=OP.mult)
                Eh = ew

                pde = nt("pde")
                veng.tensor_tensor(pde[:], A[6], A[7], op=OP.add)
                TH = nt("TH", 7)
                Th = [TH[:, h * T:(h + 1) * T] for h in range(7)]
                act = nc.scalar.activation
                act(Th[0], A[0], AF.Tanh, scale=-0.5, bias=CB[0.5])
                act(Th[1], A[1], AF.Tanh, scale=-0.5, bias=CB[0.25])
                act(Th[2], A[2], AF.Tanh, scale=-0.5, bias=CB[0.15])
                act(Th[3], A[3], AF.Tanh, scale=-0.5, bias=CB[0.1])
                act(Th[4], A[4], AF.Tanh, scale=0.5, bias=CB[-0.25])
                act(Th[5], A[5], AF.Tanh, scale=-0.5, bias=CB[-0.25])
                act(Th[6], pde[:], AF.Tanh, scale=0.5, bias=CB[-0.5])

                # R2_h = w_h * (tanh+1) * E_h  == 2 * w_h * risk_h
                # (host divides each block by 2*w_h when unpacking)
                R2 = opool.tile([128, 7 * T], HDT, tag="r2")
                for h in range(7):
                    rh = R2[:, h * T:(h + 1) * T]
                    veng.tensor_scalar(rh, Th[h], 1.0, None, op0=OP.add)
                    veng.tensor_tensor(rh, rh, Eh[h], op=OP.mult)
                # TOT = sum_h R2_h  (= 2 * total_risk)
                TOT = opool.tile([128, T], HDT, tag="tot")
                veng.tensor_tensor(TOT[:], R2[:, 0:T], R2[:, T:2 * T],
                                   op=OP.add)
                for h in range(2, 7):
                    veng.tensor_tensor(TOT[:], TOT[:],
                                       R2[:, h * T:(h + 1) * T],
                                       op=OP.add)
                nc.sync.dma_start(out=outr[:, roff:roff + 7 * T], in_=R2[:])
                nc.sync.dma_start(out=outt[:, toff:toff + T], in_=TOT[:])
                roff += 7 * T
                toff += T
    nc.compile()
    return nc


def kernel(embedding, W1, b1, W2, b2, Wh, bh, g2_weights):
    import sys
    sys.path.insert(0, "/opt/trn_rl_repo")
    from concourse.bass_utils import run_bass_kernel_spmd

    embedding = np.asarray(embedding, np.float32)
    W1 = np.asarray(W1, np.float32)
    b1 = np.asarray(b1, np.float32)
    W2 = np.asarray(W2, np.float32)
    b2 = np.asarray(b2, np.float32)
    Wh = np.asarray(Wh, np.float32)
    bh = np.asarray(bh, np.float32)
    g2 = np.asarray(g2_weights, np.float64)

    e = np.exp(g2 - g2.max())
    wsoft = (e / e.sum()).astype(np.float64)

    weff = (W2 @ Wh).astype(np.float32)            # [64, 29]
    beta = (b2 @ Wh + bh).astype(np.float32)       # [29]
    # permuted/duplicated/scaled blocks
    wcols = np.stack([weff[:, c] * s for c, s in BLOCKS], axis=1)  # [64,NCOPY]
    bvec = np.array([beta[c] * s for c, s in BLOCKS], np.float32)  # [NCOPY]
    weffbd = np.zeros((128, 2 * NCOPY), np.float32)
    weffbd[0:64, 0:NCOPY] = wcols
    weffbd[64:128, NCOPY:2 * NCOPY] = wcols
    w1bd = np.zeros((128, 128), np.float32)
    w1bd[0:64, 0:64] = W1
    w1bd[64:128, 64:128] = W1
    b1bd = np.concatenate([b1, b1]).reshape(128, 1).astype(np.float32)
    # beta tile matching pc layout: col = m*(2NB) + s*NB + b
    brow = np.repeat(bvec, GPC)                      # col = b*GPC + u
    betat = np.broadcast_to(brow, (128, brow.size)).copy()

    nc = _build_bass(wsoft)

    if MM_BF16:
        import ml_dtypes
        mdt = ml_dtypes.bfloat16
    else:
        mdt = np.float32

    in_maps = []
    for i in range(NCORE):
        shard = embedding[i * BC:(i + 1) * BC]          # [125000, 64]
        xt = np.ascontiguousarray(shard.T)              # [64, 125000]
        x2 = np.zeros((128, NCOLS), mdt)
        x2[0:64, 0:HALF] = xt[:, 0:HALF].astype(mdt)
        x2[64:128, 0:HALF] = xt[:, HALF:2 * HALF].astype(mdt)
        cb = np.broadcast_to(np.array([0.5, 0.25, 0.15, 0.1, -0.25, -0.5],
                                      np.float32), (128, 6)).copy()
        in_maps.append({"x2": x2, "w1bd": w1bd.astype(mdt),
                        "b1bd": b1bd, "weffbd": weffbd.astype(mdt),
                        "betat": betat, "cbias": cb})

    import os
    trace = bool(os.environ.get("BASS_KERNEL_TRACE"))
    tmpdir = os.environ.get("BASS_KERNEL_TRACE_DIR") or None
    res = run_bass_kernel_spmd(nc, in_maps, list(range(NCORE)),
                               trace=trace, tmpdir=tmpdir)
    if trace:
        print(f"HW exec time: {res.exec_time_ns} ns", flush=True)
    results = res.results

    total = np.empty((B,), np.float32)
    risk = np.empty((B, 7), np.float32)
    for i in range(NCORE):
        outr = np.asarray(results[i]["outr"], np.float32)   # [128, 7*NG]
        outt = np.asarray(results[i]["outt"], np.float32)    # [128, NG]
        # reassemble groups: global group u (within sub-batch segment):
        # segment b covers chunks, inside: u = ci*GPC + m*2 + s
        rv = np.empty((2, NCOLS, 7), np.float32)   # [half, col, head]
        tv = np.empty((2, NCOLS), np.float32)
        roff = 0
        toff = 0
        gch = 0
        for sb_chunks in SUBS:
            T = sb_chunks * GPC
            seg = outr[:, roff:roff + 7 * T].reshape(128, 7, sb_chunks,
                                                     GPC // 2, 2)
            segt = outt[:, toff:toff + T].reshape(128, sb_chunks, GPC // 2, 2)
            # row within half = (gch+ci)*CHUNK + m*128 + q  ; half = s
            seg = seg.transpose(4, 2, 3, 0, 1)    # [s, ci, m, q, h]
            segt = segt.transpose(3, 1, 2, 0)     # [s, ci, m, q]
            ncols_seg = sb_chunks * CHUNK
            c0 = gch * CHUNK
            rv[:, c0:c0 + ncols_seg] = seg.reshape(2, ncols_seg, 7)
            tv[:, c0:c0 + ncols_seg] = segt.reshape(2, ncols_seg)
            roff += 7 * T
            toff += T
            gch += sb_chunks
        r0 = i * BC
        wsc = (0.5 / wsoft.astype(np.float32))[None, :]
        risk[r0:r0 + HALF] = rv[0, :HALF] * wsc
        risk[r0 + HALF:r0 + BC] = rv[1, :HALF] * wsc
        total[r0:r0 + HALF] = tv[0, :HALF] * 0.5
        total[r0 + HALF:r0 + BC] = tv[1, :HALF] * 0.5
    return total, risk


# revision 29
# speedup vs baseline: 1.1913x; 1.1913x over previous
"""Trainium2 Bass kernel for AnalyticalCatastropheDetector.

Strategy (8-core pure data parallel):
 - Host: transpose each batch shard to feature-major, stack two halves on
   the partition axis -> x2 [128, NCOLS].  All device DMAs are contiguous.
 - mm1: block-diag(W1,W1) stationary, x2 moving -> h1 stacked [128, N].
 - gelu on ACT (bias b1 fused), full 128 lanes.
 - mm2: g-tile stationary, block-diag Weff moving -> C row-major, where
   Weff = W2 @ Wh (folded, gelu-linear), columns permuted/duplicated into
   31 "blocks" ordered so head math uses contiguous mega-ops.
 - PSUM->SBUF copy deinterleaves C into block-major layout and adds the
   folded bias beta = b2 @ Wh + bh (one scalar_tensor_tensor per chunk).
 - Head math: DVE STT/TT chains + one mega Exp + 7 Tanh on ACT
   (sigmoid(v) = (1+tanh(v/2))/2, outputs scaled by 2, host divides).
   All ACT funcs in phase 2 live in the exp_and_others table set ->
   exactly one table switch after the gelu phase.
"""

import numpy as np
import os

MM_BF16 = os.environ.get("K_MM_BF16", "1") == "1"
HEAD_BF16 = os.environ.get("K_HEAD_BF16", "1") == "1"

B = 1_000_000
D = 64
NCORE = 8
BC = B // NCORE          # 125000 rows per core
HALF = BC // 2           # 62500
CHUNK = 1024             # x2 columns per chunk (= 2048 rows)
NCH = (HALF + CHUNK - 1) // CHUNK          # 62 chunks
NCOLS = NCH * CHUNK                        # 63488 (padded)
NB = 37                  # csb blocks: 30 copied + 7 computed Z
GPC = 2 * (CHUNK // 128)                   # 16 groups (of 128 rows) per chunk
NG = NCH * GPC                             # 992 groups per core
SUBS = [38, 24]                            # chunks per head sub-batch
assert sum(SUBS) == NCH

# Original coefficient indices (order in the reference's 29-col C):
# fold a=0 b=1 | cusp a=2 b=3 c=4 | swal a=5 b=6 c=7 d=8
# butt a=9 b=10 c=11 d=12 e=13 | hyp a=14 b=15 c=16 | ell a=19 b=20 c=21
# par a=24 b=25 c=26 d=27 e=28   (17,18,22,23 unused)
# Block table: (source_coeff, scale) per block position.
# pos 0-9:  plain-abs group; pos 10-16: square group; pos 17-30: raw group.
S3 = 3.0 ** (-1.0 / 3.0)
S10 = 10.0 ** (-1.0 / 3.0)
SB2 = 1.125 ** 0.5
# copied blocks 0-32 (ABS 0-8 | SQ 9-15 | RAW 16-29 | Zcopied 30-32);
# computed Z blocks 33-39 have no weff column.
BLOCKS = [
    (1, 1.0),            # 0  fold_b   (sig)
    (4, 1.0),            # 1  cusp_c   (sig)
    (8, 1.0),            # 2  swal_d   (sig)
    (13, 1.0),           # 3  butt_e   (sig)
    (14, 1.0),           # 4  hyp_a    (sig)
    (19, 1.0),           # 5  ell_a    (sig)
    (27, 1.0),           # 6  par_d    (sig)
    (28, 1.0),           # 7  par_e    (sig)
    (3, 3.0),            # 8  cusp_3b  -> |3b|
    (2, 1.0),            # 9  cusp_a   (sq)
    (5, 1.0),            # 10 swal_a   (sq)
    (9, S3),             # 11 butt_a3  (sq + raw cube factor)
    (10, SB2),           # 12 butt_bsq (sq)
    (14, S10),           # 13 hyp_a10  (sq + raw cube factor)
    (19, S10),           # 14 ell_a10  (sq + raw cube factor)
    (26, 0.5),           # 15 par_c2   (sq)
    (6, -4.0),           # 16 swal_bm4
    (9, 1.0),            # 17 butt_a
    (10, 1.0 / 6.0),     # 18 butt_b6
    (11, 1.0 / 3.0),     # 19 butt_c3
    (11, -1.5),          # 20 butt_c15
    (15, -2.7),          # 21 hyp_b27
    (16, 1.0),           # 22 hyp_c
    (20, 2.7),           # 23 ell_b27
    (21, 1.0),           # 24 ell_c
    (24, 1.0),           # 25 par_a
    (25, 1.0),           # 26 par_b
    (0, 1.0),            # 27 fold_a   (Z0)
    (7, 1.0),            # 28 swal_c   (Z1)
    (12, 1.0 / 3.0),     # 29 butt_d3  (Z2)
]
NCOPY = len(BLOCKS)  # 30
assert NCOPY == 30


def _build_bass(wsoft):
    import sys
    sys.path.insert(0, "/opt/trn_rl_repo")
    from concourse import bass, bacc, mybir
    from concourse.tile import TileContext

    F32 = mybir.dt.float32
    BF16 = mybir.dt.bfloat16
    MDT = BF16 if MM_BF16 else F32
    HDT = BF16 if HEAD_BF16 else F32
    AF = mybir.ActivationFunctionType
    OP = mybir.AluOpType

    nc = bacc.Bacc()
    x2 = nc.declare_dram_parameter("x2", [128, NCOLS], MDT, isOutput=False)
    w1bd = nc.declare_dram_parameter("w1bd", [128, 128], MDT, isOutput=False)
    b1bd = nc.declare_dram_parameter("b1bd", [128, 1], F32, isOutput=False)
    weffbd = nc.declare_dram_parameter("weffbd", [128, 2 * NCOPY], MDT, isOutput=False)
    betat = nc.declare_dram_parameter("betat", [128, GPC * NCOPY], F32, isOutput=False)
    cbias = nc.declare_dram_parameter("cbias", [128, 6], F32, isOutput=False)
    outr = nc.declare_dram_parameter("outr", [128, 7 * NG], HDT, isOutput=True)
    outt = nc.declare_dram_parameter("outt", [128, NG], HDT, isOutput=True)

    w = [float(v) for v in wsoft]

    with TileContext(nc) as tc:
        with (
            tc.tile_pool(name="const", bufs=1) as cpool,
            tc.tile_pool(name="xin", bufs=8) as xpool,
            tc.tile_pool(name="g", bufs=4) as gpool,
            tc.tile_pool(name="p1", bufs=2, space="PSUM") as p1pool,
            tc.tile_pool(name="pc", bufs=4, space="PSUM") as pcpool,
            tc.tile_pool(name="csb", bufs=2) as csbpool,
            tc.tile_pool(name="scr", bufs=1) as scr,
            tc.tile_pool(name="outp", bufs=2) as opool,
        ):
            w1t = cpool.tile([128, 128], MDT, tag="w1")
            nc.sync.dma_start(out=w1t[:], in_=w1bd[:])
            b1t = cpool.tile([128, 1], F32, tag="b1")
            nc.sync.dma_start(out=b1t[:], in_=b1bd[:])
            wet = cpool.tile([128, 2 * NCOPY], MDT, tag="weff")
            nc.sync.dma_start(out=wet[:], in_=weffbd[:])
            bet = cpool.tile([128, GPC * NCOPY], F32, tag="beta")
            nc.sync.dma_start(out=bet[:], in_=betat[:])
            cbt = cpool.tile([128, 6], F32, tag="cbias")
            nc.sync.dma_start(out=cbt[:], in_=cbias[:])
            CB = {0.5: cbt[:, 0:1], 0.25: cbt[:, 1:2], 0.15: cbt[:, 2:3],
                  0.1: cbt[:, 3:4], -0.25: cbt[:, 4:5], -0.5: cbt[:, 5:6]}

            roff = 0  # running col offset into outr
            toff = 0
            gch = 0   # global chunk index
            GPS = os.environ.get("K_GPS", "0") == "1"
            veng = nc.gpsimd if GPS else nc.vector
            for sb_chunks in SUBS:
                T = sb_chunks * GPC
                csb = csbpool.tile([128, NB * T], HDT, tag="csb")
                c3 = csb[:].rearrange("p (b t) -> p b t", b=NB)
                bsrc = bet[:].rearrange("p (b u) -> p b u", b=NCOPY)
                for ci in range(sb_chunks):
                    xoff = gch * CHUNK
                    xt = xpool.tile([128, CHUNK], MDT, tag="xt")
                    nc.sync.dma_start(out=xt[:], in_=x2[:, xoff:xoff + CHUNK])
                    p1 = p1pool.tile([128, CHUNK], F32, tag="p1")
                    for q5 in range(CHUNK // 512):
                        nc.tensor.matmul(p1[:, q5 * 512:(q5 + 1) * 512],
                                         w1t[:], xt[:, q5 * 512:(q5 + 1) * 512],
                                         start=True, stop=True)
                    gt = gpool.tile([128, CHUNK], MDT, tag="g")
                    nc.scalar.activation(gt[:], p1[:], AF.Gelu,
                                         bias=b1t[:, 0:1], scale=1.0)
                    pc = pcpool.tile([128, GPC * NCOPY], F32, tag="pc")
                    pv = pc[:].rearrange("p (b u) -> p b u", b=NCOPY)
                    for m in range(GPC // 2):
                        nc.tensor.matmul(
                            pv[:, :, m * 2:(m + 1) * 2].rearrange(
                                "p b s -> p s b"),
                            gt[:, m * 128:(m + 1) * 128],
                            wet[:], start=True, stop=True)
                    dst = c3[:, 0:NCOPY, ci * GPC:(ci + 1) * GPC]
                    nc.vector.scalar_tensor_tensor(
                        dst, pv, 1.0, bsrc, op0=OP.mult, op1=OP.add)
                    gch += 1

                # ---- head math over csb [128, NB*T] ----
                def blk(i):
                    return csb[:, i * T:(i + 1) * T]

                def nt(tagname, width=1):
                    return scr.tile([128, width * T], HDT, tag=tagname,
                                    name=tagname)

                stt = nc.vector.scalar_tensor_tensor
                tt = nc.vector.tensor_tensor
                ts = nc.vector.tensor_scalar

                X = [blk(i) for i in range(NB)]
                ABS = nt("ABS", 9)
                nc.scalar.activation(ABS[:], csb[:, 0:9 * T], AF.Abs)
                A = [ABS[:, i * T:(i + 1) * T] for i in range(9)]
                SQ = nt("SQ", 7)
                tt(SQ[:], csb[:, 9 * T:16 * T], csb[:, 9 * T:16 * T],
                   op=OP.mult)
                Q = [SQ[:, i * T:(i + 1) * T] for i in range(7)]

                # computed Z blocks (30-36) inside csb
                tt(X[30], Q[0], A[8], op=OP.subtract)       # cusp: a^2-3|b|
                tt(X[31], Q[1], X[16], op=OP.add)           # swal: a^2-4b
                t1 = nt("t1")
                tt(t1[:], X[17], X[18], op=OP.mult)         # ab/6
                tt(X[32], X[19], t1[:], op=OP.subtract)     # butt T2
                t2 = nt("t2")
                tt(t2[:], Q[2], X[11], op=OP.mult)          # a^3/3
                t3 = nt("t3")
                tt(t3[:], X[17], X[20], op=OP.mult)         # -1.5ac
                tt(t2[:], t2[:], t3[:], op=OP.add)
                tt(X[33], t2[:], Q[3], op=OP.add)           # butt T4
                t4 = nt("t4")
                tt(t4[:], Q[4], X[13], op=OP.mult)          # a^3/10
                t5 = nt("t5")
                tt(t5[:], X[21], X[22], op=OP.mult)         # -2.7bc
                tt(X[34], t4[:], t5[:], op=OP.add)          # hyp z
                tt(t4[:], Q[5], X[14], op=OP.mult)
                tt(t5[:], X[23], X[24], op=OP.mult)         # +2.7bc
                tt(X[35], t4[:], t5[:], op=OP.add)          # ell z
                tt(t1[:], X[25], X[26], op=OP.mult)         # ab
                tt(X[36], t1[:], Q[6], op=OP.subtract)      # par z

                Zv = csb[:, 27 * T:37 * T]
                ZA = nt("ZA", 10)
                stt(ZA[:], Zv, -1.0, Zv, op0=OP.mult, op1=OP.max)
                EX = nt("EX", 10)
                nc.scalar.activation(EX[:], ZA[:], AF.Exp, scale=-1.0)
                e = [EX[:, i * T:(i + 1) * T] for i in range(10)]
                # weighted E'_h = w_h * (product of exp factors)
                EW = nt("EW", 7)
                ew = [EW[:, i * T:(i + 1) * T] for i in range(7)]
                ts(ew[0], e[0], w[0], None, op0=OP.mult)
                ts(ew[1], e[3], w[1], None, op0=OP.mult)
                stt(ew[2], e[1], w[2], e[4], op0=OP.mult, op1=OP.mult)
                stt(ew[3], e[2], w[3], e[5], op0=OP.mult, op1=OP.mult)
                tt(ew[3], ew[3], e[6], op=OP.mult)
                ts(ew[4], e[7], w[4], None, op0=OP.mult)
                ts(ew[5], e[8], w[5], None, op0=OP.mult)
                ts(ew[6], e[9], w[6], None, op0=OP.mult)
                Eh = ew

                pde = nt("pde")
                veng.tensor_tensor(pde[:], A[6], A[7], op=OP.add)
                TH = nt("TH", 7)
                Th = [TH[:, h * T:(h + 1) * T] for h in range(7)]
                act = nc.scalar.activation
                act(Th[0], A[0], AF.Tanh, scale=-0.5, bias=CB[0.5])
                act(Th[1], A[1], AF.Tanh, scale=-0.5, bias=CB[0.25])
                act(Th[2], A[2], AF.Tanh, scale=-0.5, bias=CB[0.15])
                act(Th[3], A[3], AF.Tanh, scale=-0.5, bias=CB[0.1])
                act(Th[4], A[4], AF.Tanh, scale=0.5, bias=CB[-0.25])
                act(Th[5], A[5], AF.Tanh, scale=-0.5, bias=CB[-0.25])
                act(Th[6], pde[:], AF.Tanh, scale=0.5, bias=CB[-0.5])

                # R2_h = w_h * (tanh+1) * E_h  == 2 * w_h * risk_h
                # (host divides each block by 2*w_h when unpacking)
                R2 = opool.tile([128, 7 * T], HDT, tag="r2")
                for h in range(7):
                    rh = R2[:, h * T:(h + 1) * T]
                    veng.tensor_scalar(rh, Th[h], 1.0, None, op0=OP.add)
                    veng.tensor_tensor(rh, rh, Eh[h], op=OP.mult)
                # TOT = sum_h R2_h  (= 2 * total_risk)
                TOT = opool.tile([128, T], HDT, tag="tot")
                veng.tensor_tensor(TOT[:], R2[:, 0:T], R2[:, T:2 * T],
                                   op=OP.add)
                for h in range(2, 7):
                    veng.tensor_tensor(TOT[:], TOT[:],
                                       R2[:, h * T:(h + 1) * T],
                                       op=OP.add)
                nc.sync.dma_start(out=outr[:, roff:roff + 7 * T], in_=R2[:])
                nc.sync.dma_start(out=outt[:, toff:toff + T], in_=TOT[:])
                roff += 7 * T
                toff += T
    nc.compile()
    return nc


def kernel(embedding, W1, b1, W2, b2, Wh, bh, g2_weights):
    import sys
    sys.path.insert(0, "/opt/trn_rl_repo")
    from concourse.bass_utils import run_bass_kernel_spmd

    embedding = np.asarray(embedding, np.float32)
    W1 = np.asarray(W1, np.float32)
    b1 = np.asarray(b1, np.float32)
    W2 = np.asarray(W2, np.float32)
    b2 = np.asarray(b2, np.float32)
    Wh = np.asarray(Wh, np.float32)
    bh = np.asarray(bh, np.float32)
    g2 = np.asarray(g2_weights, np.float64)

    e = np.exp(g2 - g2.max())
    wsoft = (e / e.sum()).astype(np.float64)

    weff = (W2 @ Wh).astype(np.float32)            # [64, 29]
    beta = (b2 @ Wh + bh).astype(np.float32)       # [29]
    # permuted/duplicated/scaled blocks
    wcols = np.stack([weff[:, c] * s for c, s in BLOCKS], axis=1)  # [64,NCOPY]
    bvec = np.array([beta[c] * s for c, s in BLOCKS], np.float32)  # [NCOPY]
    weffbd = np.zeros((128, 2 * NCOPY), np.float32)
    weffbd[0:64, 0:NCOPY] = wcols
    weffbd[64:128, NCOPY:2 * NCOPY] = wcols
    w1bd = np.zeros((128, 128), np.float32)
    w1bd[0:64, 0:64] = W1
    w1bd[64:128, 64:128] = W1
    b1bd = np.concatenate([b1, b1]).reshape(128, 1).astype(np.float32)
    # beta tile matching pc layout: col = m*(2NB) + s*NB + b
    brow = np.repeat(bvec, GPC)                      # col = b*GPC + u
    betat = np.broadcast_to(brow, (128, brow.size)).copy()

    nc = _build_bass(wsoft)

    if MM_BF16:
        import ml_dtypes
        mdt = ml_dtypes.bfloat16
    else:
        mdt = np.float32

    in_maps = []
    for i in range(NCORE):
        shard = embedding[i * BC:(i + 1) * BC]          # [125000, 64]
        xt = np.ascontiguousarray(shard.T)              # [64, 125000]
        x2 = np.zeros((128, NCOLS), mdt)
        x2[0:64, 0:HALF] = xt[:, 0:HALF].astype(mdt)
        x2[64:128, 0:HALF] = xt[:, HALF:2 * HALF].astype(mdt)
        cb = np.broadcast_to(np.array([0.5, 0.25, 0.15, 0.1, -0.25, -0.5],
                                      np.float32), (128, 6)).copy()
        in_maps.append({"x2": x2, "w1bd": w1bd.astype(mdt),
                        "b1bd": b1bd, "weffbd": weffbd.astype(mdt),
                        "betat": betat, "cbias": cb})

    import os
    trace = bool(os.environ.get("BASS_KERNEL_TRACE"))
    tmpdir = os.environ.get("BASS_KERNEL_TRACE_DIR") or None
    res = run_bass_kernel_spmd(nc, in_maps, list(range(NCORE)),
                               trace=trace, tmpdir=tmpdir)
    if trace:
        print(f"HW exec time: {res.exec_time_ns} ns", flush=True)
    results = res.results

    total = np.empty((B,), np.float32)
    risk = np.empty((B, 7), np.float32)
    for i in range(NCORE):
        outr = np.asarray(results[i]["outr"], np.float32)   # [128, 7*NG]
        outt = np.asarray(results[i]["outt"], np.float32)    # [128, NG]
        # reassemble groups: global group u (within sub-batch segment):
        # segment b covers chunks, inside: u = ci*GPC + m*2 + s
        rv = np.empty((2, NCOLS, 7), np.float32)   # [half, col, head]
        tv = np.empty((2, NCOLS), np.float32)
        roff = 0
        toff = 0
        gch = 0
        for sb_chunks in SUBS:
            T = sb_chunks * GPC
            seg = outr[:, roff:roff + 7 * T].reshape(128, 7, sb_chunks,
                                                     GPC // 2, 2)
            segt = outt[:, toff:toff + T].reshape(128, sb_chunks, GPC // 2, 2)
            # row within half = (gch+ci)*CHUNK + m*128 + q  ; half = s
            seg = seg.transpose(4, 2, 3, 0, 1)    # [s, ci, m, q, h]
            segt = segt.transpose(3, 1, 2, 0)     # [s, ci, m, q]
            ncols_seg = sb_chunks * CHUNK
            c0 = gch * CHUNK
            rv[:, c0:c0 + ncols_seg] = seg.reshape(2, ncols_seg, 7)
            tv[:, c0:c0 + ncols_seg] = segt.reshape(2, ncols_seg)
            roff += 7 * T
            toff += T
            gch += sb_chunks
        r0 = i * BC
        wsc = (0.5 / wsoft.astype(np.float32))[None, :]
        risk[r0:r0 + HALF] = rv[0, :HALF] * wsc
        risk[r0 + HALF:r0 + BC] = rv[1, :HALF] * wsc
        total[r0:r0 + HALF] = tv[0, :HALF] * 0.5
        total[r0 + HALF:r0 + BC] = tv[1, :HALF] * 0.5
    return total, risk
